# revision 1
# baseline (speedup 1.0000x reference)
"""BitStackLinear Trainium2 kernel.

Computes out = x @ w.T where w = sum_i sign_i * (u_i @ vt_i), signs unpacked
from 4 packed bit-planes (one byte = 8 signs, little-endian).

Strategy: tensor-parallel over out_features across 8 NeuronCores
(1376 rows each). Per core, on device:

  Phase R (reconstruct w.T shard [4096, 1376] into DRAM, per 128-row k-slab):
    - PE: r_i = vt_i.T @ u_i.T (rank-16 fp32r matmuls) -> PSUM
    - ScalarE: r2_i = psum->SBUF copy with per-partition scale 2^(1-j), j=p%8
    - DMA: packed sign bytes broadcast 8x across partitions
    - GpSimd: a_i = bytes & (1<<j)  in {0, 2^j}
    - DVE: t_i = (a_i - 2^(j-1)) * r2_i = sign_i * r_i ; acc += t_i
  Phase G (GEMM out.T = w.T^T-contraction, fp32r):
    - x.T chunk [4096, 1024] resident in SBUF (moving operand)
    - w.T tiles streamed from DRAM once per m-block (stationary operand)
    - PSUM accumulation over k (32 x 128), ScalarE evacuation, DMA out

kernel(**inputs) takes the full unsharded inputs and returns the full output.
Host work is layout only: transposes, dtype reinterpretation, sharding.
"""

import numpy as np

import concourse.bass as bass
import concourse.bacc as bacc
import concourse.mybir as mybir
import concourse.tile as tile

W_BIT = 4
OUT_F = 11008
IN_F = 4096
RANK = 16
NCORES = 8
O_SHARD = OUT_F // NCORES          # 1376
O_TILES = (O_SHARD + 127) // 128   # 11 (last tile 96 wide)
K_TILES = IN_F // 128              # 32
MB = 1024                          # m-block (resident x.T chunk width)


def _bitstack_body(tc, aps, M):
    nc = tc.nc
    xT, qbT, uT, vt, bm, hm, pps, wt_d, outT = (
        aps["xT"], aps["qbT"], aps["uT"], aps["vt"], aps["bm"], aps["hm"],
        aps["pps"], aps["wt_d"], aps["outT"],
    )
    f32, u8, i32 = mybir.dt.float32, mybir.dt.uint8, mybir.dt.int32
    f32r = mybir.dt.float32r
    n_mb = M // MB

    import contextlib
    with contextlib.ExitStack() as ctx:
        pool = ctx.enter_context(tc.tile_pool(name="sb", bufs=1))
        psum = ctx.enter_context(tc.tile_pool(name="ps", bufs=2, space="PSUM"))

        # ---- constants resident in SBUF ----
        bm_t = pool.tile([128, O_SHARD], u8, name="bm_t")
        nc.sync.dma_start(bm_t, bm)
        hm_t = pool.tile([128, 1], f32, name="hm_t")
        nc.sync.dma_start(hm_t, hm)
        pps_t = pool.tile([128, 1], f32, name="pps_t")
        nc.sync.dma_start(pps_t, pps)
        # prefetch m-block 0's x chunk during recon (no deps on recon)
        xk0 = []
        for k in range(K_TILES):
            t = pool.tile([128, MB], f32r, name=f"xk0_{k}", tag="xk", bufs=34)
            nc.sync.dma_start(t, xT[k * 128:(k + 1) * 128, 0:MB].bitcast(f32r))
            xk0.append(t)

        # ---- Phase R: reconstruct w.T k-slabs into wt_d ----
        for ks in range(K_TILES):
            acc = pool.tile([128, O_SHARD], f32, name=f"acc{ks}", tag="acc", bufs=2)
            for i in range(W_BIT):
                # vt slice [16, 128] and u.T [16, O] for this (slab, bit)
                vtb = pool.tile([16, 128], f32r, name=f"vtb{ks}_{i}", tag="vtb", bufs=4)
                nc.sync.dma_start(vtb, vt[i, :, ks * 128:(ks + 1) * 128].bitcast(f32r))
                utb = pool.tile([16, O_SHARD], f32r, name=f"utb{ks}_{i}", tag="utb", bufs=2)
                nc.sync.dma_start(utb, uT[i].bitcast(f32r))
                # r_i = vt_i.T @ u_i.T -> psum chunks (single-bank tiles), then
                # r2 = psum -> sbuf with per-partition scale 2^(1-j)
                r2 = pool.tile([128, O_SHARD], f32, name=f"r2_{ks}_{i}", tag="r2", bufs=2)
                for ci, c0 in enumerate(range(0, O_SHARD, 512)):
                    c1 = min(c0 + 512, O_SHARD)
                    pr = psum.tile([128, 512], f32, name=f"pr{ks}_{i}_{ci}", tag="ps", bufs=6)
                    nc.tensor.matmul(
                        pr[:, :c1 - c0], vtb,
                        utb[:, c0:c1],
                        start=True, stop=True,
                    )
                    nc.scalar.activation(r2[:, c0:c1], pr[:, :c1 - c0],
                                         mybir.ActivationFunctionType.Copy,
                                         scale=pps_t)
                # packed bytes, broadcast 8x along partitions
                bts = pool.tile([128, O_SHARD], u8, name=f"bts{ks}_{i}", tag="bts", bufs=2)
                src = qbT[i, ks * 16:(ks + 1) * 16][:, None, :].to_broadcast(
                    (16, 8, O_SHARD))
                nc.sync.dma_start(bts, src)
                # a = bytes & bitmask -> {0, 2^j}; AND runs on DVE over int32
                # views (4 packed bytes/lane/cycle; bitwise ops are DVE+i32 only)
                a_t = pool.tile([128, O_SHARD], u8, name=f"a{ks}_{i}", tag="a", bufs=2)
                nc.vector.tensor_tensor(out=a_t.bitcast(i32), in0=bts.bitcast(i32),
                                        in1=bm_t.bitcast(i32),
                                        op=mybir.AluOpType.bitwise_and)
                # t = (a - 2^(j-1)) * r2 = sign * r  (DVE); accumulate on GpSimd
                if i == 0:
                    nc.vector.scalar_tensor_tensor(
                        out=acc, in0=a_t, scalar=hm_t, in1=r2,
                        op0=mybir.AluOpType.subtract, op1=mybir.AluOpType.mult)
                else:
                    t_t = pool.tile([128, O_SHARD], f32, name=f"t{ks}_{i}", tag="tt",
                                    bufs=1)
                    nc.vector.scalar_tensor_tensor(
                        out=t_t, in0=a_t, scalar=hm_t, in1=r2,
                        op0=mybir.AluOpType.subtract, op1=mybir.AluOpType.mult)
                    nc.vector.tensor_tensor(out=acc, in0=acc, in1=t_t,
                                            op=mybir.AluOpType.add)
            # store slab to wt_d[ot][:, ks, :]
            for ot in range(O_TILES):
                ow = min(128, O_SHARD - ot * 128)
                nc.sync.dma_start(wt_d[ot, :, ks, :ow],
                                  acc[:, ot * 128:ot * 128 + ow])

        # ---- Phase G: out.T[o, m] = sum_k wT[k, o] * xT[k, m] ----
        for mb in range(n_mb):
            if mb == 0:
                xk = xk0
            else:
                xk = []
                for k in range(K_TILES):
                    t = pool.tile([128, MB], f32r, name=f"xk{mb}_{k}", tag="xk",
                                  bufs=34)
                    nc.sync.dma_start(t, xT[k * 128:(k + 1) * 128,
                                            mb * MB:(mb + 1) * MB].bitcast(f32r))
                    xk.append(t)
            for ot in range(O_TILES):
                ow = min(128, O_SHARD - ot * 128)
                # stream w.T k-column for this o-tile in two halves
                wc = []
                for kh in range(2):
                    t = pool.tile([128, 16, 128], f32r, name=f"wc{mb}_{ot}_{kh}",
                                  tag="wc", bufs=2)
                    nc.sync.dma_start(t[:, :, :ow],
                                      wt_d[ot, :, kh * 16:(kh + 1) * 16, :ow]
                                      .bitcast(f32r))
                    wc.append(t)
                # two 512-m psum groups accumulated together; halves
                # interleaved per k so consecutive matmuls share the same
                # stationary tile (walrus ldw-opt dedups the reload)
                nh = MB // 512
                pss = [psum.tile([128, 512], f32, name=f"g{mb}_{ot}_{h}",
                                 tag="ps", bufs=6) for h in range(nh)]
                for k in range(K_TILES):
                    for h in range(nh):
                        nc.tensor.matmul(
                            pss[h][:ow],
                            wc[k // 16][:, k % 16, :ow],
                            xk[k][:, h * 512:(h + 1) * 512],
                            start=(k == 0), stop=(k == K_TILES - 1),
                        )
                for h in range(nh):
                    ost = pool.tile([128, 512], f32, name=f"ost{mb}_{ot}_{h}",
                                    tag="ost", bufs=2)
                    nc.scalar.copy(ost[:ow], pss[h][:ow])
                    nc.sync.dma_start(
                        outT[ot * 128:ot * 128 + ow,
                             mb * MB + h * 512: mb * MB + (h + 1) * 512],
                        ost[:ow])


def build_bass(M=8192):
    nc = bacc.Bacc("TRN2", target_bir_lowering=False, debug=False)
    f32, u8 = mybir.dt.float32, mybir.dt.uint8
    aps = {}
    aps["xT"] = nc.dram_tensor("xT", [IN_F, M], f32, kind="ExternalInput").ap()
    aps["qbT"] = nc.dram_tensor("qbT", [W_BIT, IN_F // 8, O_SHARD], u8,
                                kind="ExternalInput").ap()
    aps["uT"] = nc.dram_tensor("uT", [W_BIT, RANK, O_SHARD], f32,
                               kind="ExternalInput").ap()
    aps["vt"] = nc.dram_tensor("vt", [W_BIT, RANK, IN_F], f32,
                               kind="ExternalInput").ap()
    aps["bm"] = nc.dram_tensor("bm", [128, O_SHARD], u8, kind="ExternalInput").ap()
    aps["hm"] = nc.dram_tensor("hm", [128, 1], f32, kind="ExternalInput").ap()
    aps["pps"] = nc.dram_tensor("pps", [128, 1], f32, kind="ExternalInput").ap()
    aps["wt_d"] = nc.dram_tensor("wt_d", [O_TILES, 128, K_TILES, 128], f32,
                                 kind="Internal").ap()
    aps["outT"] = nc.dram_tensor("outT", [O_SHARD, M], f32,
                                 kind="ExternalOutput").ap()
    with tile.TileContext(nc) as tc:
        _bitstack_body(tc, aps, M)
    nc.compile()
    return nc


def prep_inputs(x, qweight, u, vt):
    """Host-side layout prep (transposes / dtype views / sharding only)."""
    M = x.shape[0] * x.shape[1]
    xT = np.ascontiguousarray(x.reshape(M, IN_F).T)
    qb = qweight.astype(np.uint8)  # values 0..255 stored in int32
    p = np.arange(128)
    bm = (np.uint8(1) << (p % 8).astype(np.uint8))[:, None] * np.ones(
        (1, O_SHARD), np.uint8)
    hm = (2.0 ** ((p % 8) - 1.0)).astype(np.float32).reshape(128, 1)
    pps = (2.0 ** (1.0 - (p % 8))).astype(np.float32).reshape(128, 1)
    vt_c = np.ascontiguousarray(vt)
    in_maps = []
    for c in range(NCORES):
        sl = slice(c * O_SHARD, (c + 1) * O_SHARD)
        qbT = np.ascontiguousarray(
            qb.reshape(W_BIT, OUT_F, IN_F // 8)[:, sl, :].transpose(0, 2, 1))
        uT = np.ascontiguousarray(u[:, sl, :].transpose(0, 2, 1))
        in_maps.append({
            "xT": xT, "qbT": qbT, "uT": uT, "vt": vt_c,
            "bm": bm, "hm": hm, "pps": pps,
        })
    return in_maps


def _enable_ldw_opt():
    """Rewrite our walrus invocation to enable redundant-LDWEIGHTS
    elimination (consecutive matmuls sharing a stationary tile skip the
    reload)."""
    from concourse import bass_utils as bu
    if getattr(bu, "_ldw_opt_patched", False):
        return
    orig = bu.run_command

    def patched(argv, **kw):
        argv = ["--enable-ldw-opt=true" if a == "--enable-ldw-opt=false" else a
                for a in argv]
        return orig(argv, **kw)

    bu.run_command = patched
    bu._ldw_opt_patched = True


def kernel(x, qweight, u, vt):
    from concourse import bass_utils
    _enable_ldw_opt()
    x = np.asarray(x)
    qweight = np.asarray(qweight)
    u = np.asarray(u)
    vt = np.asarray(vt)
    B, S, _ = x.shape
    M = B * S
    nc = build_bass(M)
    in_maps = prep_inputs(x, qweight, u, vt)
    res = bass_utils.run_bass_kernel_spmd(nc, in_maps, core_ids=list(range(NCORES)))
    out = np.empty((M, OUT_F), np.float32)
    for c in range(NCORES):
        out[:, c * O_SHARD:(c + 1) * O_SHARD] = res.results[c]["outT"].T
    return out.reshape(B, S, OUT_F)


if __name__ == "__main__":
    # smoke test at small M via CoreSim is in sim_test.py; here run full HW
    rng = np.random.default_rng(0)
    x = rng.standard_normal((4, 2048, IN_F)).astype(np.float32)
    qw = rng.integers(0, 256, size=(W_BIT, OUT_F * IN_F // 8)).astype(np.int32)
    uu = (rng.standard_normal((W_BIT, OUT_F, RANK)) * 0.05).astype(np.float32)
    vv = (rng.standard_normal((W_BIT, RANK, IN_F)) * 0.05).astype(np.float32)
    out = kernel(x=x, qweight=qw, u=uu, vt=vv)
    print(out.shape, out.dtype)



# revision 8
# speedup vs baseline: 1.2085x; 1.2085x over previous
"""BitStackLinear Trainium2 kernel (v2: fp16 SBUF-resident weights).

Computes out = x @ w.T where w = sum_i sign_i * (u_i @ vt_i), signs unpacked
from 4 packed bit-planes (one byte = 8 signs, little-endian).

Tensor-parallel over out_features across 8 NeuronCores (1376 cols each).
Per core:

  Prep: vts = vt * 2^(1-(k%8)) cast to fp16 (via DRAM), u.T cast to fp16.
  Recon (per 128-row k-slab, per o-half [512 | 864]):
    PE:  r = vts_i.T @ u16_i  -> PSUM (rank-16 fp16 matmuls)
    DVE: a = bytes & bitmask (i32), t = (a - 2^(j-1)) * r (STT from PSUM,
         fp16 out; the 2^(1-j) descale is pre-folded into vts), acc += t.
    w.T slabs live in SBUF as fp16 [128, 512] + [128, 864] per k-slab.
  GEMM (two passes over o-halves; pass 1 overlaps recon of half 2):
    stationary = x.T tile [128k, 128m] fp16 (DMA f32 + DVE cast),
    moving     = w.T slab [128k, ow] fp16 from SBUF,
    PSUM [128m, ow] accumulated over 32 k-slabs, ScalarE evac, DMA out
    as out[m, o] row-major (no transpose on host).

kernel(**inputs) takes the full unsharded inputs, returns the full output.
Host work is layout only: transposes, dtype reinterpretation, sharding.
"""

import numpy as np

import concourse.bass as bass
import concourse.bacc as bacc
import concourse.mybir as mybir
import concourse.tile as tile

W_BIT = 4
OUT_F = 11008
IN_F = 4096
RANK = 16
NCORES = 8
O_SHARD = OUT_F // NCORES          # 1376
K_TILES = IN_F // 128              # 32
OH1 = 512                          # o-half 1 (GEMM pass 1, 1 psum bank)
OH2 = O_SHARD - OH1                # 864 = 512 + 352
H2CH = [(0, 512), (512, 352)]      # h2 psum chunks (relative to OH1)


def _body(tc, aps, M):
    nc = tc.nc
    xT, qbT, uT, vt, pps, bm, hm, vts_d, out = (
        aps["xT"], aps["qbT"], aps["uT"], aps["vt"], aps["pps"], aps["bm"],
        aps["hm"], aps["vts_d"], aps["out"])
    f32, f16, u8, i32 = (mybir.dt.float32, mybir.dt.float16, mybir.dt.uint8,
                         mybir.dt.int32)
    NMT = M // 128

    import contextlib
    with contextlib.ExitStack() as ctx:
        pool = ctx.enter_context(tc.tile_pool(name="sb", bufs=1))
        psum = ctx.enter_context(tc.tile_pool(name="ps", bufs=1, space="PSUM"))

        # ---- constants ----
        bm_t = pool.tile([128, O_SHARD], u8, name="bm_t")
        nc.sync.dma_start(bm_t, bm)
        bm32 = bm_t.bitcast(i32)
        hm_t = pool.tile([128, 1], f32, name="hm_t")
        nc.sync.dma_start(hm_t, hm)

        # ---- vts_d = vt * 2^(1-(k%8)) as fp16, via DRAM ----
        pps_t = pool.tile([16, 512], f32, name="pps_t")
        nc.sync.dma_start(pps_t, pps)
        for i in range(W_BIT):
            for c in range(8):
                vstg = pool.tile([16, 512], f32, name=f"vstg{i}_{c}",
                                 tag="vstg", bufs=2)
                nc.sync.dma_start(vstg, vt[i, :, c * 512:(c + 1) * 512])
                vs16 = pool.tile([16, 512], f16, name=f"vs16{i}_{c}",
                                 tag="vs16", bufs=2)
                nc.vector.tensor_tensor(out=vs16, in0=vstg, in1=pps_t,
                                        op=mybir.AluOpType.mult)
                nc.sync.dma_start(vts_d[i, :, c * 512:(c + 1) * 512], vs16)
        # ---- ut16 resident [16, 4, O_SHARD] fp16 ----
        ut16 = pool.tile([16, W_BIT, O_SHARD], f16, name="ut16")
        for i in range(W_BIT):
            for c, (c0, cw) in enumerate([(0, 688), (688, 688)]):
                ustg = pool.tile([16, 688], f32, name=f"ustg{i}_{c}",
                                 tag="ustg", bufs=2)
                nc.sync.dma_start(ustg, uT[i, :, c0:c0 + cw])
                nc.vector.tensor_copy(ut16[:, i, c0:c0 + cw], ustg)

        # ---- persistent w.T halves ----
        w1 = [pool.tile([128, OH1], f16, name=f"w1_{ks}", tag="w1",
                        bufs=K_TILES) for ks in range(K_TILES)]
        w2 = [pool.tile([128, OH2], f16, name=f"w2_{ks}", tag="w2",
                        bufs=K_TILES) for ks in range(K_TILES)]

        def recon(ks, half):
            o0, ow = (0, OH1) if half == 1 else (OH1, OH2)
            wt = w1[ks] if half == 1 else w2[ks]
            chunks = [(0, OH1)] if half == 1 else H2CH
            vtb = pool.tile([16, W_BIT, 128], f16, name=f"vtb{ks}_{half}",
                            tag="vtb", bufs=3)
            src = vts_d.rearrange("i r k -> r i k")
            nc.sync.dma_start(vtb, src[:, :, ks * 128:(ks + 1) * 128])
            for i in range(W_BIT):
                bts = pool.tile([128, ow], u8, name=f"bts{ks}_{half}_{i}",
                                tag=f"bts{half}", bufs=2)
                src = qbT[i, ks * 16:(ks + 1) * 16, o0:o0 + ow]
                nc.sync.dma_start(bts, src[:, None, :].to_broadcast(
                    (16, 8, ow)))
                a_t = pool.tile([128, ow], u8, name=f"a{ks}_{half}_{i}",
                                tag=f"a{half}", bufs=2)
                nc.vector.tensor_tensor(out=a_t.bitcast(i32),
                                        in0=bts.bitcast(i32),
                                        in1=bm32[:, 0:ow // 4],
                                        op=mybir.AluOpType.bitwise_and)
                t16 = None
                if i > 0:
                    t16 = pool.tile([128, ow], f16, name=f"t{ks}_{half}_{i}",
                                    tag=f"t{half}", bufs=2)
                for (c0, cw) in chunks:
                    pr = psum.tile([128, cw], f32, name=f"pr{ks}_{half}_{i}_{c0}",
                                   tag=f"rp{cw}", bufs=2)
                    nc.tensor.matmul(pr, vtb[:, i, :],
                                     ut16[:, i, o0 + c0:o0 + c0 + cw],
                                     start=True, stop=True)
                    dst = wt if i == 0 else t16
                    nc.vector.scalar_tensor_tensor(
                        out=dst[:, c0:c0 + cw], in0=a_t[:, c0:c0 + cw],
                        scalar=hm_t, in1=pr,
                        op0=mybir.AluOpType.subtract, op1=mybir.AluOpType.mult)
                if i > 0:
                    nc.vector.tensor_tensor(out=wt, in0=wt, in1=t16,
                                            op=mybir.AluOpType.add)

        # ---- recon half 1 (o columns 0:512) ----
        for ks in range(K_TILES):
            recon(ks, 1)

        # ---- GEMM passes ----
        def gemm_pass(o0, ow, chunks, wlist, interleave_recon):
            x8 = [None] * NMT
            x16 = [None] * NMT

            def dma_x(mt):
                t = pool.tile([128, K_TILES, 128], f32, name=f"x8_{o0}_{mt}",
                              tag="x8", bufs=3)
                src = xT.rearrange("(k p) m -> p k m", k=K_TILES, p=128)
                nc.sync.dma_start(t, src[:, :, mt * 128:(mt + 1) * 128])
                x8[mt] = t

            def cast_x(mt):
                t = pool.tile([128, K_TILES, 128], f16, name=f"x16_{o0}_{mt}",
                              tag="x16", bufs=2)
                nc.vector.tensor_copy(
                    t.rearrange("p k m -> p (k m)"),
                    x8[mt].rearrange("p k m -> p (k m)"))
                x16[mt] = t

            dma_x(0)
            dma_x(1)
            cast_x(0)
            for mt in range(NMT):
                if mt + 2 < NMT:
                    dma_x(mt + 2)
                if mt + 1 < NMT:
                    cast_x(mt + 1)
                pgs = []
                for (c0, cw) in chunks:
                    pg = psum.tile([128, cw], f32, name=f"pg{o0}_{mt}_{c0}",
                                   tag=f"pg{cw}", bufs=2)
                    pgs.append((pg, c0, cw))
                for ks in range(K_TILES):
                    for (pg, c0, cw) in pgs:
                        nc.tensor.matmul(pg, x16[mt][:, ks, :],
                                         wlist[ks][:, c0:c0 + cw],
                                         start=(ks == 0),
                                         stop=(ks == K_TILES - 1))
                for (pg, c0, cw) in pgs:
                    ost = pool.tile([128, cw], f32, name=f"ost{o0}_{mt}_{c0}",
                                    tag=f"ost{cw}", bufs=2)
                    nc.scalar.copy(ost, pg)
                    nc.sync.dma_start(
                        out[mt * 128:(mt + 1) * 128, o0 + c0:o0 + c0 + cw],
                        ost)
                if interleave_recon and mt < K_TILES:
                    recon(mt, 2)

        gemm_pass(0, OH1, [(0, OH1)], w1, True)
        gemm_pass(OH1, OH2, H2CH, w2, False)


def build_bass(M=8192):
    nc = bacc.Bacc("TRN2", target_bir_lowering=False, debug=False)
    f32, f16, u8 = mybir.dt.float32, mybir.dt.float16, mybir.dt.uint8
    aps = {}
    aps["xT"] = nc.dram_tensor("xT", [IN_F, M], f32, kind="ExternalInput").ap()
    aps["qbT"] = nc.dram_tensor("qbT", [W_BIT, IN_F // 8, O_SHARD], u8,
                                kind="ExternalInput").ap()
    aps["uT"] = nc.dram_tensor("uT", [W_BIT, RANK, O_SHARD], f32,
                               kind="ExternalInput").ap()
    aps["vt"] = nc.dram_tensor("vt", [W_BIT, RANK, IN_F], f32,
                               kind="ExternalInput").ap()
    aps["pps"] = nc.dram_tensor("pps", [16, 512], f32,
                                kind="ExternalInput").ap()
    aps["bm"] = nc.dram_tensor("bm", [128, O_SHARD], u8,
                               kind="ExternalInput").ap()
    aps["hm"] = nc.dram_tensor("hm", [128, 1], f32, kind="ExternalInput").ap()
    aps["vts_d"] = nc.dram_tensor("vts_d", [W_BIT, RANK, IN_F], f16,
                                  kind="Internal").ap()
    aps["out"] = nc.dram_tensor("out", [M, O_SHARD], f32,
                                kind="ExternalOutput").ap()
    with tile.TileContext(nc) as tc:
        _body(tc, aps, M)
    nc.compile()
    return nc


def prep_inputs(x, qweight, u, vt):
    """Host-side layout prep (transposes / dtype views / sharding only)."""
    M = x.shape[0] * x.shape[1]
    xT = np.ascontiguousarray(x.reshape(M, IN_F).T)
    qb = qweight.astype(np.uint8)  # values 0..255 stored in int32
    p = np.arange(128)
    bm = (np.uint8(1) << (p % 8).astype(np.uint8))[:, None] * np.ones(
        (1, O_SHARD), np.uint8)
    hm = (2.0 ** ((p % 8) - 1.0)).astype(np.float32).reshape(128, 1)
    pps = np.tile((2.0 ** (1.0 - (np.arange(512) % 8))).astype(np.float32),
                  (16, 1))
    vt_c = np.ascontiguousarray(vt)
    in_maps = []
    for c in range(NCORES):
        sl = slice(c * O_SHARD, (c + 1) * O_SHARD)
        qbT = np.ascontiguousarray(
            qb.reshape(W_BIT, OUT_F, IN_F // 8)[:, sl, :].transpose(0, 2, 1))
        uT = np.ascontiguousarray(u[:, sl, :].transpose(0, 2, 1))
        in_maps.append({
            "xT": xT, "qbT": qbT, "uT": uT, "vt": vt_c,
            "pps": pps, "bm": bm, "hm": hm,
        })
    return in_maps


def assemble(results, M):
    out = np.empty((M, OUT_F), np.float32)
    for c in range(NCORES):
        out[:, c * O_SHARD:(c + 1) * O_SHARD] = results[c]["out"]
    return out


def _enable_ldw_opt():
    """No-op: fp16 LDWEIGHTS (~53ns, FWL) fully overlaps matmuls via the PE
    reorder window; walrus ldw-opt is both unnecessary and incompatible with
    the fp16 ldweights this kernel emits."""


def kernel(x, qweight, u, vt):
    from concourse import bass_utils
    _enable_ldw_opt()
    x = np.asarray(x)
    qweight = np.asarray(qweight)
    u = np.asarray(u)
    vt = np.asarray(vt)
    B, S, _ = x.shape
    M = B * S
    nc = build_bass(M)
    in_maps = prep_inputs(x, qweight, u, vt)
    res = bass_utils.run_bass_kernel_spmd(nc, in_maps,
                                          core_ids=list(range(NCORES)))
    return assemble(res.results, M).reshape(B, S, OUT_F)


if __name__ == "__main__":
    rng = np.random.default_rng(0)
    x = rng.standard_normal((4, 2048, IN_F)).astype(np.float32)
    qw = rng.integers(0, 256, size=(W_BIT, OUT_F * IN_F // 8)).astype(np.int32)
    uu = (rng.standard_normal((W_BIT, OUT_F, RANK)) * 0.05).astype(np.float32)
    vv = (rng.standard_normal((W_BIT, RANK, IN_F)) * 0.05).astype(np.float32)
    out = kernel(x=x, qweight=qw, u=uu, vt=vv)
    print(out.shape, out.dtype)


# revision 11
# speedup vs baseline: 1.2421x; 1.0279x over previous
"""BitStackLinear Trainium2 kernel (v2: fp16 SBUF-resident weights).

Computes out = x @ w.T where w = sum_i sign_i * (u_i @ vt_i), signs unpacked
from 4 packed bit-planes (one byte = 8 signs, little-endian).

Tensor-parallel over out_features across 8 NeuronCores (1376 cols each).
Per core:

  Prep: vts = vt * 2^(1-(k%8)) cast to fp16 (via DRAM), u.T cast to fp16.
  Recon (per 128-row k-slab, per o-half [512 | 864]):
    PE:  r = vts_i.T @ u16_i  -> PSUM (rank-16 fp16 matmuls)
    DVE: a = bytes & bitmask (i32), t = (a - 2^(j-1)) * r (STT from PSUM,
         fp16 out; the 2^(1-j) descale is pre-folded into vts), acc += t.
    w.T slabs live in SBUF as fp16 [128, 512] + [128, 864] per k-slab.
  GEMM (two passes over o-halves; pass 1 overlaps recon of half 2):
    stationary = x.T tile [128k, 128m] fp16 (DMA f32 + DVE cast),
    moving     = w.T slab [128k, ow] fp16 from SBUF,
    PSUM [128m, ow] accumulated over 32 k-slabs, ScalarE evac, DMA out
    as out[m, o] row-major (no transpose on host).

kernel(**inputs) takes the full unsharded inputs, returns the full output.
Host work is layout only: transposes, dtype reinterpretation, sharding.
"""

import numpy as np

import concourse.bass as bass
import concourse.bacc as bacc
import concourse.mybir as mybir
import concourse.tile as tile

W_BIT = 4
OUT_F = 11008
IN_F = 4096
RANK = 16
NCORES = 8
O_SHARD = OUT_F // NCORES          # 1376
K_TILES = IN_F // 128              # 32
OH1 = 512                          # o-half 1 (GEMM pass 1, 1 psum bank)
OH2 = O_SHARD - OH1                # 864 = 512 + 352
H2CH = [(0, 512), (512, 352)]      # h2 psum chunks (relative to OH1)


def _body(tc, aps, M):
    nc = tc.nc
    xT, qbE, uT, vt, pps, bm, hm, vts_d, out1, out2 = (
        aps["xT"], aps["qbE"], aps["uT"], aps["vt"], aps["pps"], aps["bm"],
        aps["hm"], aps["vts_d"], aps["out1"], aps["out2"])
    f32, f16, u8, i32 = (mybir.dt.float32, mybir.dt.float16, mybir.dt.uint8,
                         mybir.dt.int32)
    NMT = M // 128

    import contextlib
    with contextlib.ExitStack() as ctx:
        pool = ctx.enter_context(tc.tile_pool(name="sb", bufs=1))
        psum = ctx.enter_context(tc.tile_pool(name="ps", bufs=1, space="PSUM"))

        # ---- constants ----
        bm_t = pool.tile([128, W_BIT, O_SHARD], u8, name="bm_t")
        nc.sync.dma_start(bm_t, bm)
        bm32 = bm_t.bitcast(i32)
        hm_t = pool.tile([128, 1], f32, name="hm_t")
        nc.sync.dma_start(hm_t, hm)

        # ---- vts_d = vt * 2^(1-(k%8)) as fp16, via DRAM ----
        pps_t = pool.tile([16, 512], f32, name="pps_t")
        nc.sync.dma_start(pps_t, pps)
        for i in range(W_BIT):
            for c in range(8):
                vstg = pool.tile([16, 512], f32, name=f"vstg{i}_{c}",
                                 tag="vstg", bufs=2)
                nc.sync.dma_start(vstg, vt[i, :, c * 512:(c + 1) * 512])
                vs16 = pool.tile([16, 512], f16, name=f"vs16{i}_{c}",
                                 tag="vs16", bufs=2)
                nc.vector.tensor_tensor(out=vs16, in0=vstg, in1=pps_t,
                                        op=mybir.AluOpType.mult)
                nc.sync.dma_start(vts_d[i, :, c * 512:(c + 1) * 512], vs16)
        # ---- ut16 resident [16, 4, O_SHARD] fp16 ----
        ut16 = pool.tile([16, W_BIT, O_SHARD], f16, name="ut16")
        for i in range(W_BIT):
            for c, (c0, cw) in enumerate([(0, 688), (688, 688)]):
                ustg = pool.tile([16, 688], f32, name=f"ustg{i}_{c}",
                                 tag="ustg", bufs=2)
                nc.sync.dma_start(ustg, uT[i, :, c0:c0 + cw])
                nc.vector.tensor_copy(ut16[:, i, c0:c0 + cw], ustg)

        # ---- persistent w.T halves ----
        w1 = [pool.tile([128, OH1], f16, name=f"w1_{ks}", tag="w1",
                        bufs=K_TILES) for ks in range(K_TILES)]
        w2 = [pool.tile([128, OH2], f16, name=f"w2_{ks}", tag="w2",
                        bufs=K_TILES) for ks in range(K_TILES)]

        def recon(ks, half):
            o0, ow = (0, OH1) if half == 1 else (OH1, OH2)
            wt = w1[ks] if half == 1 else w2[ks]
            chunks = [(0, OH1)] if half == 1 else H2CH
            vtb = pool.tile([16, W_BIT, 128], f16, name=f"vtb{ks}_{half}",
                            tag="vtb", bufs=3)
            src = vts_d.rearrange("i r k -> r i k")
            nc.sync.dma_start(vtb, src[:, :, ks * 128:(ks + 1) * 128])
            bts = pool.tile([128, W_BIT, ow], u8, name=f"bts{ks}_{half}",
                            tag=f"bts{half}", bufs=2)
            nc.sync.dma_start(bts, qbE[ks, :, :, o0:o0 + ow])
            a4 = pool.tile([128, W_BIT, ow], u8, name=f"a{ks}_{half}",
                           tag=f"a{half}", bufs=2)
            nc.vector.tensor_tensor(out=a4.bitcast(i32),
                                    in0=bts.bitcast(i32),
                                    in1=bm32[:, :, o0 // 4:(o0 + ow) // 4],
                                    op=mybir.AluOpType.bitwise_and)
            for i in range(W_BIT):
                t16 = None
                if i > 0:
                    t16 = pool.tile([128, ow], f16, name=f"t{ks}_{half}_{i}",
                                    tag=f"t{half}", bufs=2)
                for (c0, cw) in chunks:
                    pr = psum.tile([128, cw], f32, name=f"pr{ks}_{half}_{i}_{c0}",
                                   tag=f"rp{cw}", bufs=2)
                    nc.tensor.matmul(pr, vtb[:, i, :],
                                     ut16[:, i, o0 + c0:o0 + c0 + cw],
                                     start=True, stop=True)
                    dst = wt if i == 0 else t16
                    nc.vector.scalar_tensor_tensor(
                        out=dst[:, c0:c0 + cw], in0=a4[:, i, c0:c0 + cw],
                        scalar=hm_t, in1=pr,
                        op0=mybir.AluOpType.subtract, op1=mybir.AluOpType.mult)
                if i > 0:
                    nc.vector.tensor_tensor(out=wt, in0=wt, in1=t16,
                                            op=mybir.AluOpType.add)

        # ---- recon half 1 (o columns 0:512) ----
        for ks in range(K_TILES):
            recon(ks, 1)

        # ---- GEMM passes ----
        def gemm_pass(o0, ow, chunks, wlist, interleave_recon, outd):
            x8 = [None] * NMT
            x16 = [None] * NMT

            def dma_x(mt):
                t = pool.tile([128, K_TILES, 128], f32, name=f"x8_{o0}_{mt}",
                              tag="x8", bufs=2)
                src = xT.rearrange("(k p) m -> p k m", k=K_TILES, p=128)
                nc.sync.dma_start(t, src[:, :, mt * 128:(mt + 1) * 128])
                x8[mt] = t

            def cast_x(mt):
                t = pool.tile([128, K_TILES, 128], f16, name=f"x16_{o0}_{mt}",
                              tag="x16", bufs=2)
                nc.vector.tensor_copy(
                    t.rearrange("p k m -> p (k m)"),
                    x8[mt].rearrange("p k m -> p (k m)"))
                x16[mt] = t

            ost_cur = [None]
            dma_x(0)
            dma_x(1)
            cast_x(0)
            for mt in range(NMT):
                if mt + 2 < NMT:
                    dma_x(mt + 2)
                if mt + 1 < NMT:
                    cast_x(mt + 1)
                pgs = []
                for (c0, cw) in chunks:
                    pg = psum.tile([128, cw], f32, name=f"pg{o0}_{mt}_{c0}",
                                   tag=f"pg{cw}", bufs=2)
                    pgs.append((pg, c0, cw))
                for ks in range(K_TILES):
                    for (pg, c0, cw) in pgs:
                        nc.tensor.matmul(pg, x16[mt][:, ks, :],
                                         wlist[ks][:, c0:c0 + cw],
                                         start=(ks == 0),
                                         stop=(ks == K_TILES - 1))
                if mt % 2 == 0:
                    ost = pool.tile([128, 2, ow], f16, name=f"ost{o0}_{mt}",
                                    tag=f"ost{o0}", bufs=2)
                    ost_cur[0] = ost
                for (pg, c0, cw) in pgs:
                    nc.scalar.copy(ost_cur[0][:, mt % 2, c0:c0 + cw], pg)
                if mt % 2 == 1:
                    dst = outd.rearrange("(t p) o -> p t o", p=128)
                    nc.scalar.dma_start(dst[:, mt - 1:mt + 1, :], ost_cur[0])
                if interleave_recon and mt < K_TILES:
                    recon(mt, 2)

        gemm_pass(0, OH1, [(0, OH1)], w1, True, out1)
        gemm_pass(OH1, OH2, H2CH, w2, False, out2)


def build_bass(M=8192):
    nc = bacc.Bacc("TRN2", target_bir_lowering=False, debug=False)
    f32, f16, u8 = mybir.dt.float32, mybir.dt.float16, mybir.dt.uint8
    aps = {}
    aps["xT"] = nc.dram_tensor("xT", [IN_F, M], f32, kind="ExternalInput").ap()
    aps["qbE"] = nc.dram_tensor("qbE", [K_TILES, 128, W_BIT, O_SHARD], u8,
                                kind="ExternalInput").ap()
    aps["uT"] = nc.dram_tensor("uT", [W_BIT, RANK, O_SHARD], f32,
                               kind="ExternalInput").ap()
    aps["vt"] = nc.dram_tensor("vt", [W_BIT, RANK, IN_F], f32,
                               kind="ExternalInput").ap()
    aps["pps"] = nc.dram_tensor("pps", [16, 512], f32,
                                kind="ExternalInput").ap()
    aps["bm"] = nc.dram_tensor("bm", [128, W_BIT, O_SHARD], u8,
                               kind="ExternalInput").ap()
    aps["hm"] = nc.dram_tensor("hm", [128, 1], f32, kind="ExternalInput").ap()
    aps["vts_d"] = nc.dram_tensor("vts_d", [W_BIT, RANK, IN_F], f16,
                                  kind="Internal").ap()
    aps["out1"] = nc.dram_tensor("out1", [M, OH1], f16,
                                 kind="ExternalOutput").ap()
    aps["out2"] = nc.dram_tensor("out2", [M, OH2], f16,
                                 kind="ExternalOutput").ap()
    with tile.TileContext(nc) as tc:
        _body(tc, aps, M)
    nc.compile()
    return nc


def prep_inputs(x, qweight, u, vt):
    """Host-side layout prep (transposes / dtype views / sharding only)."""
    M = x.shape[0] * x.shape[1]
    xT = np.ascontiguousarray(x.reshape(M, IN_F).T)
    qb = qweight.astype(np.uint8)  # values 0..255 stored in int32
    p = np.arange(128)
    bm = (np.uint8(1) << (p % 8).astype(np.uint8))[:, None, None] * np.ones(
        (1, W_BIT, O_SHARD), np.uint8)
    hm = (2.0 ** ((p % 8) - 1.0)).astype(np.float32).reshape(128, 1)
    pps = np.tile((2.0 ** (1.0 - (np.arange(512) % 8))).astype(np.float32),
                  (16, 1))
    vt_c = np.ascontiguousarray(vt)
    in_maps = []
    for c in range(NCORES):
        sl = slice(c * O_SHARD, (c + 1) * O_SHARD)
        qbT = qb.reshape(W_BIT, OUT_F, IN_F // 8)[:, sl, :].transpose(0, 2, 1)
        # expand to [K_TILES, 128, W_BIT, O_SHARD]: byte replicated 8x along
        # partitions (pure layout: repeat + transpose)
        qbE = np.ascontiguousarray(
            np.repeat(qbT.reshape(W_BIT, K_TILES, 16, O_SHARD), 8,
                      axis=2).transpose(1, 2, 0, 3))
        uT = np.ascontiguousarray(u[:, sl, :].transpose(0, 2, 1))
        in_maps.append({
            "xT": xT, "qbE": qbE, "uT": uT, "vt": vt_c,
            "pps": pps, "bm": bm, "hm": hm,
        })
    return in_maps


def assemble(results, M):
    out = np.empty((M, OUT_F), np.float32)
    for c in range(NCORES):
        out[:, c * O_SHARD:c * O_SHARD + OH1] = results[c]["out1"]
        out[:, c * O_SHARD + OH1:(c + 1) * O_SHARD] = results[c]["out2"]
    return out


def _enable_ldw_opt():
    """No-op: fp16 LDWEIGHTS (~53ns, FWL) fully overlaps matmuls via the PE
    reorder window; walrus ldw-opt is both unnecessary and incompatible with
    the fp16 ldweights this kernel emits."""


def kernel(x, qweight, u, vt):
    from concourse import bass_utils
    _enable_ldw_opt()
    x = np.asarray(x)
    qweight = np.asarray(qweight)
    u = np.asarray(u)
    vt = np.asarray(vt)
    B, S, _ = x.shape
    M = B * S
    nc = build_bass(M)
    in_maps = prep_inputs(x, qweight, u, vt)
    res = bass_utils.run_bass_kernel_spmd(nc, in_maps,
                                          core_ids=list(range(NCORES)))
    return assemble(res.results, M).reshape(B, S, OUT_F)


if __name__ == "__main__":
    rng = np.random.default_rng(0)
    x = rng.standard_normal((4, 2048, IN_F)).astype(np.float32)
    qw = rng.integers(0, 256, size=(W_BIT, OUT_F * IN_F // 8)).astype(np.int32)
    uu = (rng.standard_normal((W_BIT, OUT_F, RANK)) * 0.05).astype(np.float32)
    vv = (rng.standard_normal((W_BIT, RANK, IN_F)) * 0.05).astype(np.float32)
    out = kernel(x=x, qweight=qw, u=uu, vt=vv)
    print(out.shape, out.dtype)


# revision 12
# speedup vs baseline: 1.6330x; 1.3147x over previous
"""BitStackLinear Trainium2 kernel (v2: fp16 SBUF-resident weights).

Computes out = x @ w.T where w = sum_i sign_i * (u_i @ vt_i), signs unpacked
from 4 packed bit-planes (one byte = 8 signs, little-endian).

Tensor-parallel over out_features across 8 NeuronCores (1376 cols each).
Per core:

  Prep: vts = vt * 2^(1-(k%8)) cast to fp16 (via DRAM), u.T cast to fp16.
  Recon (per 128-row k-slab, per o-half [512 | 864]):
    PE:  r = vts_i.T @ u16_i  -> PSUM (rank-16 fp16 matmuls)
    DVE: a = bytes & bitmask (i32), t = (a - 2^(j-1)) * r (STT from PSUM,
         fp16 out; the 2^(1-j) descale is pre-folded into vts), acc += t.
    w.T slabs live in SBUF as fp16 [128, 512] + [128, 864] per k-slab.
  GEMM (two passes over o-halves; pass 1 overlaps recon of half 2):
    stationary = x.T tile [128k, 128m] fp16 (DMA f32 + DVE cast),
    moving     = w.T slab [128k, ow] fp16 from SBUF,
    PSUM [128m, ow] accumulated over 32 k-slabs, ScalarE evac, DMA out
    as out[m, o] row-major (no transpose on host).

kernel(**inputs) takes the full unsharded inputs, returns the full output.
Host work is layout only: transposes, dtype reinterpretation, sharding.
"""

import numpy as np

import concourse.bass as bass
import concourse.bacc as bacc
import concourse.mybir as mybir
import concourse.tile as tile

W_BIT = 4
OUT_F = 11008
IN_F = 4096
RANK = 16
NCORES = 8
O_SHARD = OUT_F // NCORES          # 1376
K_TILES = IN_F // 128              # 32
OH1 = 512                          # o-half 1 (GEMM pass 1, 1 psum bank)
OH2 = O_SHARD - OH1                # 864 = 512 + 352
H2CH = [(0, 512), (512, 352)]      # h2 psum chunks (relative to OH1)


def _body(tc, aps, M):
    nc = tc.nc
    x5d, qbE1, qbE2, uT, vt, pps, bm, hm, vts_d, out1, out2 = (
        aps["x5d"], aps["qbE1"], aps["qbE2"], aps["uT"], aps["vt"],
        aps["pps"], aps["bm"], aps["hm"], aps["vts_d"], aps["out1"],
        aps["out2"])
    f32, f16, u8, i32 = (mybir.dt.float32, mybir.dt.float16, mybir.dt.uint8,
                         mybir.dt.int32)
    NMT = M // 128

    import contextlib
    with contextlib.ExitStack() as ctx:
        pool = ctx.enter_context(tc.tile_pool(name="sb", bufs=1))
        psum = ctx.enter_context(tc.tile_pool(name="ps", bufs=1, space="PSUM"))

        # ---- constants ----
        bm_t = pool.tile([128, W_BIT, O_SHARD], u8, name="bm_t")
        nc.sync.dma_start(bm_t, bm)
        bm32 = bm_t.bitcast(i32)
        hm_t = pool.tile([128, 1], f32, name="hm_t")
        nc.sync.dma_start(hm_t, hm)

        # ---- vts_d = vt * 2^(1-(k%8)) as fp16, via DRAM ----
        pps_t = pool.tile([16, 512], f32, name="pps_t")
        nc.sync.dma_start(pps_t, pps)
        for i in range(W_BIT):
            for c in range(8):
                vstg = pool.tile([16, 512], f32, name=f"vstg{i}_{c}",
                                 tag="vstg", bufs=2)
                nc.sync.dma_start(vstg, vt[i, :, c * 512:(c + 1) * 512])
                vs16 = pool.tile([16, 512], f16, name=f"vs16{i}_{c}",
                                 tag="vs16", bufs=2)
                nc.vector.tensor_tensor(out=vs16, in0=vstg, in1=pps_t,
                                        op=mybir.AluOpType.mult)
                nc.sync.dma_start(vts_d[i, :, c * 512:(c + 1) * 512], vs16)
        # ---- ut16 resident [16, 4, O_SHARD] fp16 ----
        ut16 = pool.tile([16, W_BIT, O_SHARD], f16, name="ut16")
        for i in range(W_BIT):
            for c, (c0, cw) in enumerate([(0, 688), (688, 688)]):
                ustg = pool.tile([16, 688], f32, name=f"ustg{i}_{c}",
                                 tag="ustg", bufs=2)
                nc.sync.dma_start(ustg, uT[i, :, c0:c0 + cw])
                nc.vector.tensor_copy(ut16[:, i, c0:c0 + cw], ustg)

        # ---- persistent w.T halves ----
        w1 = [pool.tile([128, OH1], f16, name=f"w1_{ks}", tag="w1",
                        bufs=K_TILES) for ks in range(K_TILES)]
        w2 = [pool.tile([128, OH2], f16, name=f"w2_{ks}", tag="w2",
                        bufs=K_TILES) for ks in range(K_TILES)]

        def recon(ks, half):
            o0, ow = (0, OH1) if half == 1 else (OH1, OH2)
            wt = w1[ks] if half == 1 else w2[ks]
            chunks = [(0, OH1)] if half == 1 else H2CH
            vtb = pool.tile([16, W_BIT, 128], f16, name=f"vtb{ks}_{half}",
                            tag="vtb", bufs=3)
            src = vts_d.rearrange("i r k -> r i k")
            nc.sync.dma_start(vtb, src[:, :, ks * 128:(ks + 1) * 128])
            bts = pool.tile([128, W_BIT, ow], u8, name=f"bts{ks}_{half}",
                            tag=f"bts{half}", bufs=2)
            nc.sync.dma_start(bts, (qbE1 if half == 1 else qbE2)[ks])
            a4 = pool.tile([128, W_BIT, ow], u8, name=f"a{ks}_{half}",
                           tag=f"a{half}", bufs=2)
            nc.vector.tensor_tensor(out=a4.bitcast(i32),
                                    in0=bts.bitcast(i32),
                                    in1=bm32[:, :, o0 // 4:(o0 + ow) // 4],
                                    op=mybir.AluOpType.bitwise_and)
            for i in range(W_BIT):
                t16 = None
                if i > 0:
                    t16 = pool.tile([128, ow], f16, name=f"t{ks}_{half}_{i}",
                                    tag=f"t{half}", bufs=2)
                for (c0, cw) in chunks:
                    pr = psum.tile([128, cw], f32, name=f"pr{ks}_{half}_{i}_{c0}",
                                   tag=f"rp{cw}", bufs=2)
                    nc.tensor.matmul(pr, vtb[:, i, :],
                                     ut16[:, i, o0 + c0:o0 + c0 + cw],
                                     start=True, stop=True)
                    dst = wt if i == 0 else t16
                    nc.vector.scalar_tensor_tensor(
                        out=dst[:, c0:c0 + cw], in0=a4[:, i, c0:c0 + cw],
                        scalar=hm_t, in1=pr,
                        op0=mybir.AluOpType.subtract, op1=mybir.AluOpType.mult)
                if i > 0:
                    nc.vector.tensor_tensor(out=wt, in0=wt, in1=t16,
                                            op=mybir.AluOpType.add)

        # ---- recon half 1 (o columns 0:512) ----
        for ks in range(K_TILES):
            recon(ks, 1)

        # ---- GEMM passes ----
        def gemm_pass(o0, ow, chunks, wlist, interleave_recon, outd):
            x8 = [None] * NMT
            x16 = [None] * NMT

            def dma_x(mt):
                t = pool.tile([128, K_TILES, 128], f32, name=f"x8_{o0}_{mt}",
                              tag="x8", bufs=2)
                eng = nc.sync if mt % 2 == 0 else nc.scalar
                eng.dma_start(t, x5d[mt])
                x8[mt] = t

            def cast_x(mt):
                t = pool.tile([128, K_TILES, 128], f16, name=f"x16_{o0}_{mt}",
                              tag="x16", bufs=2)
                nc.vector.tensor_copy(
                    t.rearrange("p k m -> p (k m)"),
                    x8[mt].rearrange("p k m -> p (k m)"))
                x16[mt] = t

            ost_cur = [None]
            dma_x(0)
            dma_x(1)
            cast_x(0)
            for mt in range(NMT):
                if mt + 2 < NMT:
                    dma_x(mt + 2)
                if mt + 1 < NMT:
                    cast_x(mt + 1)
                pgs = []
                for (c0, cw) in chunks:
                    pg = psum.tile([128, cw], f32, name=f"pg{o0}_{mt}_{c0}",
                                   tag=f"pg{cw}", bufs=2)
                    pgs.append((pg, c0, cw))
                for ks in range(K_TILES):
                    for (pg, c0, cw) in pgs:
                        nc.tensor.matmul(pg, x16[mt][:, ks, :],
                                         wlist[ks][:, c0:c0 + cw],
                                         start=(ks == 0),
                                         stop=(ks == K_TILES - 1))
                if mt % 2 == 0:
                    ost = pool.tile([128, 2, ow], f16, name=f"ost{o0}_{mt}",
                                    tag=f"ost{o0}", bufs=2)
                    ost_cur[0] = ost
                for (pg, c0, cw) in pgs:
                    nc.scalar.copy(ost_cur[0][:, mt % 2, c0:c0 + cw], pg)
                if mt % 2 == 1:
                    dst = outd.rearrange("(t p) o -> p t o", p=128)
                    nc.scalar.dma_start(dst[:, mt - 1:mt + 1, :], ost_cur[0])
                if interleave_recon and mt < K_TILES:
                    recon(mt, 2)

        gemm_pass(0, OH1, [(0, OH1)], w1, True, out1)
        gemm_pass(OH1, OH2, H2CH, w2, False, out2)


def build_bass(M=8192):
    nc = bacc.Bacc("TRN2", target_bir_lowering=False, debug=False)
    f32, f16, u8 = mybir.dt.float32, mybir.dt.float16, mybir.dt.uint8
    aps = {}
    aps["x5d"] = nc.dram_tensor("x5d", [M // 128, 128, K_TILES, 128], f32,
                                kind="ExternalInput").ap()
    aps["qbE1"] = nc.dram_tensor("qbE1", [K_TILES, 128, W_BIT, OH1], u8,
                                 kind="ExternalInput").ap()
    aps["qbE2"] = nc.dram_tensor("qbE2", [K_TILES, 128, W_BIT, OH2], u8,
                                 kind="ExternalInput").ap()
    aps["uT"] = nc.dram_tensor("uT", [W_BIT, RANK, O_SHARD], f32,
                               kind="ExternalInput").ap()
    aps["vt"] = nc.dram_tensor("vt", [W_BIT, RANK, IN_F], f32,
                               kind="ExternalInput").ap()
    aps["pps"] = nc.dram_tensor("pps", [16, 512], f32,
                                kind="ExternalInput").ap()
    aps["bm"] = nc.dram_tensor("bm", [128, W_BIT, O_SHARD], u8,
                               kind="ExternalInput").ap()
    aps["hm"] = nc.dram_tensor("hm", [128, 1], f32, kind="ExternalInput").ap()
    aps["vts_d"] = nc.dram_tensor("vts_d", [W_BIT, RANK, IN_F], f16,
                                  kind="Internal").ap()
    aps["out1"] = nc.dram_tensor("out1", [M, OH1], f16,
                                 kind="ExternalOutput").ap()
    aps["out2"] = nc.dram_tensor("out2", [M, OH2], f16,
                                 kind="ExternalOutput").ap()
    with tile.TileContext(nc) as tc:
        _body(tc, aps, M)
    nc.compile()
    return nc


def prep_inputs(x, qweight, u, vt):
    """Host-side layout prep (transposes / dtype views / sharding only)."""
    M = x.shape[0] * x.shape[1]
    # x5d[mt, p, k, m] = x[mt*128 + m, k*128 + p] (pure transpose/tiling)
    x5d = np.ascontiguousarray(
        x.reshape(M // 128, 128, K_TILES, 128).transpose(0, 3, 2, 1))
    qb = qweight.astype(np.uint8)  # values 0..255 stored in int32
    p = np.arange(128)
    bm = (np.uint8(1) << (p % 8).astype(np.uint8))[:, None, None] * np.ones(
        (1, W_BIT, O_SHARD), np.uint8)
    hm = (2.0 ** ((p % 8) - 1.0)).astype(np.float32).reshape(128, 1)
    pps = np.tile((2.0 ** (1.0 - (np.arange(512) % 8))).astype(np.float32),
                  (16, 1))
    vt_c = np.ascontiguousarray(vt)
    in_maps = []
    for c in range(NCORES):
        sl = slice(c * O_SHARD, (c + 1) * O_SHARD)
        qbT = qb.reshape(W_BIT, OUT_F, IN_F // 8)[:, sl, :].transpose(0, 2, 1)
        # expand to [K_TILES, 128, W_BIT, O_SHARD]: byte replicated 8x along
        # partitions (pure layout: repeat + transpose), split by o-half so
        # each DMA reads per-partition-contiguous runs
        qbE = np.repeat(qbT.reshape(W_BIT, K_TILES, 16, O_SHARD), 8,
                        axis=2).transpose(1, 2, 0, 3)
        qbE1 = np.ascontiguousarray(qbE[:, :, :, :OH1])
        qbE2 = np.ascontiguousarray(qbE[:, :, :, OH1:])
        uT = np.ascontiguousarray(u[:, sl, :].transpose(0, 2, 1))
        in_maps.append({
            "x5d": x5d, "qbE1": qbE1, "qbE2": qbE2, "uT": uT, "vt": vt_c,
            "pps": pps, "bm": bm, "hm": hm,
        })
    return in_maps


def assemble(results, M):
    out = np.empty((M, OUT_F), np.float32)
    for c in range(NCORES):
        out[:, c * O_SHARD:c * O_SHARD + OH1] = results[c]["out1"]
        out[:, c * O_SHARD + OH1:(c + 1) * O_SHARD] = results[c]["out2"]
    return out


def _enable_ldw_opt():
    """No-op: fp16 LDWEIGHTS (~53ns, FWL) fully overlaps matmuls via the PE
    reorder window; walrus ldw-opt is both unnecessary and incompatible with
    the fp16 ldweights this kernel emits."""


def kernel(x, qweight, u, vt):
    from concourse import bass_utils
    _enable_ldw_opt()
    x = np.asarray(x)
    qweight = np.asarray(qweight)
    u = np.asarray(u)
    vt = np.asarray(vt)
    B, S, _ = x.shape
    M = B * S
    nc = build_bass(M)
    in_maps = prep_inputs(x, qweight, u, vt)
    res = bass_utils.run_bass_kernel_spmd(nc, in_maps,
                                          core_ids=list(range(NCORES)))
    return assemble(res.results, M).reshape(B, S, OUT_F)


if __name__ == "__main__":
    rng = np.random.default_rng(0)
    x = rng.standard_normal((4, 2048, IN_F)).astype(np.float32)
    qw = rng.integers(0, 256, size=(W_BIT, OUT_F * IN_F // 8)).astype(np.int32)
    uu = (rng.standard_normal((W_BIT, OUT_F, RANK)) * 0.05).astype(np.float32)
    vv = (rng.standard_normal((W_BIT, RANK, IN_F)) * 0.05).astype(np.float32)
    out = kernel(x=x, qweight=qw, u=uu, vt=vv)
    print(out.shape, out.dtype)
